# revision 9
# baseline (speedup 1.0000x reference)
"""Trainium2 Bass kernel for 4-layer cross-stencil CNN (fp8 DoubleRow).

Per-core: one image [6,256,256] (batch sharded across 8 cores).

Numerics: activations and weights carried as compensated fp8 pairs
(hi = fp8(v), lo = fp8(v - hi)); every conv runs on the PE in fp8
DoubleRow mode (2 K-tiles per matmul, 0.5 cycles/output-row), giving a
near-exact product at ~4x the fp32r tap rate:

  L1: x packed as K=60 (5 pre-shifted tap groups x 6ch, hi@0-29 lo@30-59),
      one stride-0 DoubleRow matmul (tiles = W_hi / W_lo)   -> 0.5 cyc/px
  L2/L3: hs buffers [128, 2(hi/lo), rows, 258] fp8; 8 DR matmuls per
      2-row chunk cover (Whi x {hi,lo}) all taps + (Wlo x hi) all taps
      + (Wlo x lo) center                                    -> 4 cyc/px
  L4: bf16 slab matmul (all 5 taps as M-slabs) + DMA slab gather +
      bf16 selector matmul with a ones-row carrying b4       -> 2 cyc/px

Splits after L1/L2: ACT produces hf = relu(ps*c + 16b) fp16; GPSIMD
copies hf->hi (fp8); DVE computes lo = hf - hi (fp8). L3 output is a
single bf16 buffer (feeds the bf16 L4). Selector outputs for 4 chunks
are stacked at PSUM partitions 0/32/64/96 so one DVE copy drains 8 rows.

Scales (all powers of 2, exact): x,h carried at 16x; per-layer weight
scale S_l puts max|w*S| in (4,8]; ACT scales unwind them.
"""

import sys

sys.path.insert(0, "/opt/trn_rl_repo")

import ml_dtypes
import numpy as np

import concourse.bacc as bacc
import concourse.mybir as mybir
from concourse.tile import TileContext
from concourse import bass_utils
from concourse.bass_types import AP

IN_C, HID_C, OUT_C = 6, 128, 6
B, H, W = 8, 256, 256
WP = W + 2
R = 32  # output rows per strip (divides H)
N_CORES = 8

f32 = mybir.dt.float32
f16 = mybir.dt.float16
bf16 = mybir.dt.bfloat16
f8 = mybir.dt.float8e4
DR = mybir.MatmulPerfMode.DoubleRow
Relu = mybir.ActivationFunctionType.Relu
Max = mybir.AluOpType.max
Sub = mybir.AluOpType.subtract

F8 = ml_dtypes.float8_e4m3

# tap order in reference weights: 0=center, 1=up (x[h-1]), 2=down, 3=left, 4=right

X30_SLOTS = R + 10   # x30 row slots (center of row r at slot r-a+5)
H1_SLOTS = R + 6     # h1 rows [a-3, b+3), slot = row-(a-3)
H2_SLOTS = R + 4     # h2 rows [a-2, b+2)
H3_SLOTS = R + 2     # h3 rows [a-1, b+1)


def _build(cs1, cs2, cs3):
    nc = bacc.Bacc("TRN2", target_bir_lowering=False)

    xhl_d = nc.dram_tensor("xhl", [2 * IN_C, H, W], f8, kind="ExternalInput")
    w1dr_d = nc.dram_tensor("w1dr", [60, 2, HID_C], f8, kind="ExternalInput")
    w2dr_d = nc.dram_tensor("w2dr", [HID_C, 8, 2, HID_C], f8, kind="ExternalInput")
    w3dr_d = nc.dram_tensor("w3dr", [HID_C, 8, 2, HID_C], f8, kind="ExternalInput")
    w4a_d = nc.dram_tensor("w4a", [HID_C, HID_C], bf16, kind="ExternalInput")
    s6_d = nc.dram_tensor("s6", [HID_C, OUT_C], bf16, kind="ExternalInput")
    b1_d = nc.dram_tensor("b1s", [HID_C], f32, kind="ExternalInput")
    b2_d = nc.dram_tensor("b2s", [HID_C], f32, kind="ExternalInput")
    b3_d = nc.dram_tensor("b3s", [HID_C], f32, kind="ExternalInput")
    ones_d = nc.dram_tensor("ones", [R, WP], bf16, kind="ExternalInput")
    y_d = nc.dram_tensor("y", [OUT_C, H, W], f32, kind="ExternalOutput")

    with TileContext(nc) as tc:
        with (
            tc.tile_pool(name="const", bufs=1) as cpool,
            tc.tile_pool(name="bufs", bufs=1) as bpool,
            tc.tile_pool(name="io", bufs=1) as iopool,
            tc.tile_pool(name="ps", bufs=1, space="PSUM") as pmain,
        ):
            # --- resident weights / biases ---
            w1dr = cpool.tile([60, 2, HID_C], f8)
            nc.sync.dma_start(out=w1dr, in_=w1dr_d[:, :, :])
            w2dr = cpool.tile([HID_C, 8, 2, HID_C], f8)
            nc.sync.dma_start(out=w2dr, in_=w2dr_d[:, :, :, :])
            w3dr = cpool.tile([HID_C, 8, 2, HID_C], f8)
            nc.sync.dma_start(out=w3dr, in_=w3dr_d[:, :, :, :])
            w4a = cpool.tile([HID_C, HID_C], bf16)
            nc.sync.dma_start(out=w4a, in_=w4a_d[:, :])
            s6 = cpool.tile([HID_C, OUT_C], bf16)
            nc.sync.dma_start(out=s6, in_=s6_d[:, :])
            b1s = cpool.tile([HID_C, 1], f32)
            nc.sync.dma_start(out=b1s, in_=b1_d[:, None])
            b2s = cpool.tile([HID_C, 1], f32)
            nc.sync.dma_start(out=b2s, in_=b2_d[:, None])
            b3s = cpool.tile([HID_C, 1], f32)
            nc.sync.dma_start(out=b3s, in_=b3_d[:, None])

            # --- persistent strip buffers ---
            x30 = bpool.tile([60, X30_SLOTS, WP], f8)
            h1s = bpool.tile([HID_C, 2, H1_SLOTS, WP], f8)
            h2s = bpool.tile([HID_C, 2, H2_SLOTS, WP], f8)
            h3b = bpool.tile([HID_C, H3_SLOTS, WP], bf16)
            t5 = bpool.tile([HID_C, R + 2, WP], bf16)
            t5s = bpool.tile([HID_C, R, WP], bf16)

            # zero cells that are read but never written
            nc.vector.memset(x30[:, 0:6, :], 0.0)
            nc.vector.memset(x30[:, :, 1:2], 0.0)
            nc.vector.memset(x30[:, :, 256:257], 0.0)
            for hs, topz in ((h1s, 3), (h2s, 2)):
                nc.vector.memset(hs[:, :, :, 0:1], 0.0)
                nc.vector.memset(hs[:, :, :, 257:258], 0.0)
                nc.vector.memset(hs[:, :, 0:topz, :], 0.0)
            nc.vector.memset(t5[:, :, 0:1], 0.0)
            nc.vector.memset(t5[:, :, 257:258], 0.0)
            nc.vector.memset(t5[:, 0:1, :], 0.0)
            nc.vector.memset(t5s[:, :, :], 0.0)
            # ones row carrying b4 through the selector matmul (partition 12
            # is not a legal engine base -> write it via DMA)
            nc.sync.dma_start(out=t5s[12:13, :, :], in_=ones_d[None, :, :])

            x30_f = x30[:, :, :]
            x30_ps = x30_f.ap[0][0]
            h1_f = h1s[:, :, :, :]
            h1_ps, h1_plane = h1_f.ap[0][0], H1_SLOTS * WP
            h2_f = h2s[:, :, :, :]
            h2_ps, h2_plane = h2_f.ap[0][0], H2_SLOTS * WP

            def l1_rhs(s, n):
                return AP(
                    x30_f.tensor, s * WP + 1,
                    [[x30_ps, 60], [0, 2], [WP, n], [1, W]],
                )

            def conv_dr(ps, po, n, w_sb, hs_t, hs_ps, plane, s):
                """8 DoubleRow matmuls: full compensated 5-tap conv chunk.
                Writes ps[:, po:po+n, :]."""
                views = (
                    (s * WP + 1, plane),        # c  (Whi x hi, Whi x lo)
                    ((s - 1) * WP + 1, plane),  # u
                    ((s + 1) * WP + 1, plane),  # d
                    (s * WP + 0, plane),        # l
                    (s * WP + 2, plane),        # r
                    (s * WP + 1, plane),        # c  (Wlo x hi, Wlo x lo)
                    ((s - 1) * WP + 1, 2 * WP), # u/d (Wlo x hi)
                    (s * WP + 0, 2),            # l/r (Wlo x hi)
                )
                out = ps[:, po : po + n, :]
                for i, (off, d1) in enumerate(views):
                    rhs = AP(
                        hs_t.tensor, off,
                        [[hs_ps, HID_C], [d1, 2], [WP, n], [1, W]],
                    )
                    nc.tensor.matmul(
                        out, w_sb[:, i, :, :], rhs,
                        start=(i == 0), stop=(i == 7), perf_mode=DR,
                    )

            def split(ps, rows, hs, d, bias, scale, hf_tag, ci):
                """hf = relu(ps*scale + bias) f16; hi = fp8(hf); lo = hf - hi."""
                hf = iopool.tile(
                    [HID_C, 4, W], f16, tag=hf_tag, bufs=3, name=hf_tag
                )
                hfv = hf[:, 0:rows, :]
                nc.scalar.activation(hfv, ps[:, 0:rows, :], Relu, bias=bias, scale=scale)
                hi = hs[:, 0, d : d + rows, 1 : 1 + W]
                lo = hs[:, 1, d : d + rows, 1 : 1 + W]
                nc.gpsimd.tensor_copy(hi, hfv)
                nc.vector.tensor_tensor(lo, hfv, hi, Sub)

            def sel_group(a, g):
                """Deferred emitters: 4 selector matmuls stacked at psum
                partitions 0/32/64/96 + one DVE drain + 4 y DMAs."""
                def emit(a=a, g=g):
                    psl = pmain.tile(
                        [HID_C, 2, W], f32, tag="psl", bufs=1, name="psl"
                    )
                    for j in range(4):
                        d = 8 * g + 2 * j
                        nc.tensor.matmul(
                            psl[32 * j : 32 * j + OUT_C, :, :],
                            s6[:, :], t5s[:, d : d + 2, 1 : 1 + W],
                            start=True, stop=True,
                            tile_position=(0, 32 * j),
                        )
                    yt = iopool.tile([HID_C, 2, W], f32, tag="yt", bufs=2, name="yt")
                    nc.vector.tensor_copy(yt, psl)
                    for j in range(4):
                        rr = a + 8 * g + 2 * j
                        nc.sync.dma_start(
                            out=y_d[:, rr : rr + 2, :],
                            in_=yt[32 * j : 32 * j + OUT_C, :, :],
                        )
                return emit

            pending = []
            for a in range(0, H, R):
                b = a + R
                last = b == H
                lo_x, hi_x = max(0, a - 4), min(H, b + 4)

                if last:
                    nc.vector.memset(x30[:, hi_x - a + 4 : X30_SLOTS, :], 0.0)
                    nc.vector.memset(h1s[:, :, H - (a - 3) : H1_SLOTS, :], 0.0)
                    nc.vector.memset(h2s[:, :, H - (a - 2) : H2_SLOTS, :], 0.0)

                # --- load x strip: 5 shifted placements x (hi, lo) ---
                o = lo_x - a
                for part, c0 in ((0, 0), (30, IN_C)):
                    src = xhl_d[c0 : c0 + IN_C, lo_x:hi_x, :]
                    nc.sync.dma_start(
                        out=x30[part + 0 : part + 6, o + 5 : hi_x - a + 5, 1 : 1 + W],
                        in_=src)
                    nc.sync.dma_start(
                        out=x30[part + 6 : part + 12, o + 6 : hi_x - a + 6, 1 : 1 + W],
                        in_=src)
                    nc.sync.dma_start(
                        out=x30[part + 12 : part + 18, o + 4 : hi_x - a + 4, 1 : 1 + W],
                        in_=src)
                    nc.sync.dma_start(
                        out=x30[part + 18 : part + 24, o + 5 : hi_x - a + 5, 2 : 2 + W],
                        in_=src)
                    nc.sync.dma_start(
                        out=x30[part + 24 : part + 30, o + 5 : hi_x - a + 5, 0:W],
                        in_=src)

                # --- L1: rows [a-3, b+3), one stride-0 DR per 2-row chunk ---
                rr = max(0, a - 3)
                hi1 = min(H, b + 3)
                ci = 0
                while rr < hi1:
                    n = min(2, hi1 - rr)
                    s = rr - a + 5
                    ps = pmain.tile([HID_C, 2, W], f32, tag="psA", bufs=3, name="psA")
                    nc.tensor.matmul(
                        ps[:, 0:n, :], w1dr[:, :, :], l1_rhs(s, n),
                        start=True, stop=True, perf_mode=DR,
                    )
                    split(ps, n, h1s, rr - (a - 3), b1s, cs1, "hf1", ci)
                    if pending and ci % 5 == 2:
                        pending.pop(0)()
                    ci += 1
                    rr += n

                # --- L2: rows [a-2, b+2), 4-row psum groups, 8 DR per chunk ---
                rr = max(0, a - 2)
                hi2 = min(H, b + 2)
                ci = 0
                while rr < hi2:
                    rows = min(4, hi2 - rr)
                    ps = pmain.tile([HID_C, 4, W], f32, tag="psB", bufs=2, name="psB")
                    for po in range(0, rows, 2):
                        n = min(2, rows - po)
                        s = (rr + po) - (a - 3)  # h1s slot of center
                        conv_dr(ps, po, n, w2dr, h1_f, h1_ps, h1_plane, s)
                    split(ps, rows, h2s, rr - (a - 2), b2s, cs2, "hf2", ci)
                    if pending:
                        pending.pop(0)()
                    ci += 1
                    rr += rows
                while pending:
                    pending.pop(0)()

                # --- L3: rows [a-1, b+1), + slab matmuls (lagged) ---
                lo4, hi4 = max(0, a - 1), min(H, b + 1)
                slab_q = []
                ci4 = 0

                def emit_slab(r0, n0, ci):
                    s = r0 - (a - 1)
                    ps = pmain.tile([HID_C, 2, W], f32, tag="psA", bufs=3, name="psA")
                    nc.tensor.matmul(
                        ps[:, 0:n0, :], w4a[:, :], h3b[:, s : s + n0, 1 : 1 + W],
                        start=True, stop=True,
                    )
                    dst = t5[:, s : s + n0, 1 : 1 + W]
                    if ci % 2 == 0:
                        nc.scalar.activation(
                            dst, ps[:, 0:n0, :], mybir.ActivationFunctionType.Identity
                        )
                    else:
                        nc.vector.tensor_copy(dst, ps[:, 0:n0, :])

                rr = lo4
                while rr < hi4:
                    rows = min(4, hi4 - rr)
                    ps = pmain.tile([HID_C, 4, W], f32, tag="psB", bufs=2, name="psB")
                    for po in range(0, rows, 2):
                        n = min(2, rows - po)
                        s = (rr + po) - (a - 2)  # h2s slot of center
                        conv_dr(ps, po, n, w3dr, h2_f, h2_ps, h2_plane, s)
                        slab_q.append((rr + po, n))
                    d = rr - (a - 1)
                    nc.scalar.activation(
                        h3b[:, d : d + rows, 1 : 1 + W], ps[:, 0:rows, :],
                        Relu, bias=b3s, scale=cs3,
                    )
                    while slab_q and slab_q[0][0] + 4 <= rr:
                        r0, n0 = slab_q.pop(0)
                        emit_slab(r0, n0, ci4)
                        ci4 += 1
                    rr += rows

                # tail slots beyond written range must be zero for the gather
                if hi4 - (a - 1) < R + 2:
                    nc.vector.memset(t5[:, hi4 - (a - 1) : R + 2, :], 0.0)
                while slab_q:
                    r0, n0 = slab_q.pop(0)
                    emit_slab(r0, n0, ci4)
                    ci4 += 1

                # --- gather: pre-shifted tap slabs into t5s (two halves) ---
                t5f = t5.rearrange("p r c -> p (r c)")
                t5sf = t5s.rearrange("p r c -> p (r c)")
                for h0, h1r in ((0, R // 2), (R // 2, R)):
                    o0, o1 = h0 * WP, h1r * WP
                    nc.sync.dma_start(
                        out=t5sf[0:6, o0:o1], in_=t5f[0:6, o0:o1])
                    nc.sync.dma_start(
                        out=t5sf[6:12, o0:o1], in_=t5f[6:12, WP + o0 : WP + o1])
                    nc.sync.dma_start(
                        out=t5sf[32:38, o0:o1],
                        in_=t5f[32:38, 2 * WP + o0 : 2 * WP + o1])
                    nc.sync.dma_start(
                        out=t5sf[64:70, o0 + 1 : o1],
                        in_=t5f[64:70, WP + o0 : WP + o1 - 1])
                    nc.sync.dma_start(
                        out=t5sf[96:102, o0 : o1 - 1],
                        in_=t5f[96:102, WP + o0 + 1 : WP + o1])

                pending = [sel_group(a, g) for g in range(R // 8)]

            while pending:
                pending.pop(0)()

    nc.finalize()
    return nc


_NC_CACHE = {}


def _q8(v):
    return np.asarray(v, np.float32).astype(F8)


def _split8(v):
    hi = _q8(v)
    lo = _q8(np.asarray(v, np.float32) - hi.astype(np.float32))
    return hi, lo


def _wscale(w):
    m = float(np.abs(w).max())
    if m == 0.0:
        return 1.0
    return float(2.0 ** np.floor(np.log2(8.0 / m)))


def _pack_inputs(x, w1, b1, w2, b2, w3, b3, w4, b4):
    x = np.asarray(x, np.float32)
    w1 = np.asarray(w1, np.float32)
    w2 = np.asarray(w2, np.float32)
    w3 = np.asarray(w3, np.float32)
    w4 = np.asarray(w4, np.float32)
    b1 = np.asarray(b1, np.float32)
    b2 = np.asarray(b2, np.float32)
    b3 = np.asarray(b3, np.float32)
    b4 = np.asarray(b4, np.float32)

    S1, S2, S3 = _wscale(w1), _wscale(w2), _wscale(w3)

    # x pair at scale 16: xhl[0:6]=hi, xhl[6:12]=lo  (per image; built later)
    # L1 weights: K=60 (tap-group g x in-ch c; hi rows 0-29, lo rows 30-59),
    # DR tile 0 = W_hi (duplicated for hi/lo input rows), tile 1 = W_lo.
    w1h, w1l = _split8(w1 * S1)
    w1dr = np.zeros((60, 2, HID_C), F8)
    for g in range(5):
        for c in range(IN_C):
            p = g * IN_C + c
            w1dr[p, 0, :] = w1h[:, c, g]
            w1dr[p + 30, 0, :] = w1h[:, c, g]
            w1dr[p, 1, :] = w1l[:, c, g]
            w1dr[p + 30, 1, :] = w1l[:, c, g]

    def pack_dr(w, S):
        wh, wl = _split8(w * S)
        whf = wh.astype(np.float32)
        wlf = wl.astype(np.float32)
        # [ic, dr_i, tile, oc]
        wd = np.zeros((HID_C, 8, 2, HID_C), np.float32)
        for i, t in enumerate((0, 1, 2, 3, 4)):  # Whi_t x (hi, lo)
            wd[:, i, 0, :] = whf[:, :, t].T
            wd[:, i, 1, :] = whf[:, :, t].T
        wd[:, 5, 0, :] = wlf[:, :, 0].T  # Wlo_c x (hi, lo)
        wd[:, 5, 1, :] = wlf[:, :, 0].T
        wd[:, 6, 0, :] = wlf[:, :, 1].T  # Wlo_u x hi@u
        wd[:, 6, 1, :] = wlf[:, :, 2].T  # Wlo_d x hi@d
        wd[:, 7, 0, :] = wlf[:, :, 3].T  # Wlo_l x hi@l
        wd[:, 7, 1, :] = wlf[:, :, 4].T  # Wlo_r x hi@r
        return wd.astype(F8)

    w2dr = pack_dr(w2, S2)
    w3dr = pack_dr(w3, S3)

    # L4 slab weights (bf16): up@0, center@6, down@32, left@64, right@96
    w4a = np.zeros((HID_C, HID_C), np.float32)
    w4a[:, 0:OUT_C] = w4[:, :, 1].T
    w4a[:, 6 : 6 + OUT_C] = w4[:, :, 0].T
    w4a[:, 32 : 32 + OUT_C] = w4[:, :, 2].T
    w4a[:, 64 : 64 + OUT_C] = w4[:, :, 3].T
    w4a[:, 96 : 96 + OUT_C] = w4[:, :, 4].T
    s6 = np.zeros((HID_C, OUT_C), np.float32)
    for base in (0, 6, 32, 64, 96):
        s6[base + np.arange(OUT_C), np.arange(OUT_C)] = 1.0
    s6[12, :] = b4  # ones-row bias inject

    common = {
        "w1dr": w1dr,
        "w2dr": w2dr,
        "w3dr": w3dr,
        "w4a": w4a.astype(ml_dtypes.bfloat16),
        "s6": s6.astype(ml_dtypes.bfloat16),
        "b1s": 16.0 * b1,
        "b2s": 16.0 * b2,
        "b3s": b3,
        "ones": np.ones((R, WP), ml_dtypes.bfloat16),
    }
    scales = (1.0 / S1, 1.0 / S2, 1.0 / (16.0 * S3))

    xh, xl = _split8(16.0 * x)  # [B, 6, H, W]
    xhl = np.concatenate([xh, xl], axis=1)  # [B, 12, H, W]
    return xhl, common, scales


def kernel(x, w1, b1, w2, b2, w3, b3, w4, b4):
    xhl, common, scales = _pack_inputs(x, w1, b1, w2, b2, w3, b3, w4, b4)
    key = scales
    if _NC_CACHE.get("key") != key:
        _NC_CACHE["nc"] = _build(*scales)
        _NC_CACHE["key"] = key
    nc = _NC_CACHE["nc"]
    in_maps = [
        dict(common, xhl=np.ascontiguousarray(xhl[i])) for i in range(N_CORES)
    ]
    res = bass_utils.run_bass_kernel_spmd(nc, in_maps, core_ids=list(range(N_CORES)))
    out = np.stack([res.results[i]["y"] for i in range(N_CORES)], axis=0)
    return out
